# revision 1
# baseline (speedup 1.0000x reference)
"""Single-head causal self-attention on 8 Trainium2 NeuronCores.

Reference computation (per batch b):
    k = x @ Wk.T ; q = x @ Wq.T ; v = x @ Wv.T
    wei = softmax(mask(q @ k.T / sqrt(H)))
    out = wei @ v

Strategy:
  - Data parallel: shard B=256 across 8 cores (32 batches each), replicate
    weights. No cross-core communication.
  - Algebraic fusion: q @ k.T = x (Wq.T Wk) x.T.  G = Wq.T @ Wk * scale is
    precomputed once on-chip (9 matmuls), which halves the per-batch q/k
    projection work (2 * T*C*H  ->  T*C*C with C==H plus amortized G).
  - Scores are computed directly in transposed layout ST[s, t] so that the
    attention output matmul can consume exp(ST) as the stationary operand
    without any per-batch transpose of the weights matrix.
  - Softmax denominator: V is augmented with a ones column, so the output
    matmul also produces r[t] = sum_s exp(ST[s,t]); the final normalization
    is a per-partition reciprocal multiply.  No max-subtraction is needed:
    logits are ~N(0,1) scaled, |logit| < ~10, exp() is safe in fp32.
  - All matmuls run as float32r (TF32-like) at 1 cycle/row (4x faster than
    fp32) with fp32 PSUM accumulation.
"""

import numpy as np

import concourse.bass as bass
import concourse.mybir as mybir
from concourse import bacc
import concourse.tile as tile
from concourse.bass_utils import run_bass_kernel_spmd
from concourse.masks import make_identity

B, T, C, H = 256, 256, 384, 384
NCORES = 8
NB = B // NCORES  # batches per core
P = 128
CC = C // P  # 3 chunks of the embedding dim
TC = T // P  # 2 chunks of the sequence dim
SCALE = float(H) ** -0.5
F32 = mybir.dt.float32
F32R = mybir.dt.float32r

USE_F32R = True
MM_DT = F32R if USE_F32R else F32


def build_bass(nb: int = NB):
    nc = bacc.Bacc(
        "TRN2",
        target_bir_lowering=False,
        debug=False,
        enable_asserts=False,
        num_devices=NCORES,
    )
    x_d = nc.dram_tensor("x", [nb, T, C], F32, kind="ExternalInput").ap()
    wk_d = nc.dram_tensor("Wk", [H, C], F32, kind="ExternalInput").ap()
    wq_d = nc.dram_tensor("Wq", [H, C], F32, kind="ExternalInput").ap()
    wv_d = nc.dram_tensor("Wv", [H, C], F32, kind="ExternalInput").ap()
    out_d = nc.dram_tensor("out", [nb, T, H], F32, kind="ExternalOutput").ap()

    with tile.TileContext(nc) as tc:
        with (
            tc.tile_pool(name="const", bufs=1) as cpool,
            tc.tile_pool(name="sb", bufs=3) as sb,
            tc.tile_pool(name="ob", bufs=4) as obp,
            tc.tile_pool(name="pt", bufs=2, space="PSUM") as ptp,
            tc.tile_pool(name="pm", bufs=5, space="PSUM") as pmp,
        ):
            ident = cpool.tile([P, P], F32, name="ident")
            make_identity(nc, ident)

            # mask[sc][p, t] = 1.0 where (s = sc*128 + p) <= t else 0.0
            masks = []
            for sc in range(TC):
                m = cpool.tile([P, T], F32, name=f"mask{sc}")
                nc.gpsimd.memset(m, 1.0)
                nc.gpsimd.affine_select(
                    out=m,
                    in_=m,
                    compare_op=mybir.AluOpType.is_ge,
                    fill=0.0,
                    base=-(sc * P),
                    channel_multiplier=-1,
                    pattern=[[1, T]],
                )
                masks.append(m)

            # Load weights (natural [H, C] layout, 3 partition chunks each)
            wq_s, wk_s, wv_s = [], [], []
            for hc in range(CC):
                for lst, src, nm in (
                    (wq_s, wq_d, "wq"),
                    (wk_s, wk_d, "wk"),
                    (wv_s, wv_d, "wv"),
                ):
                    t_ = cpool.tile([P, C], F32, name=f"{nm}{hc}")
                    nc.sync.dma_start(t_, src[hc * P : (hc + 1) * P, :])
                    lst.append(t_)

            # G = (Wq.T @ Wk) * SCALE   tiles: [c1 partition chunk, c2 free]
            g_s = []
            for c1 in range(CC):
                pg = pmp.tile([P, 512], F32, name="pg", tag="pm")[:, :C]
                for hc in range(CC):
                    nc.tensor.matmul(
                        pg,
                        lhsT=wq_s[hc][:, c1 * P : (c1 + 1) * P],
                        rhs=wk_s[hc],
                        start=(hc == 0),
                        stop=(hc == CC - 1),
                    )
                g_t = cpool.tile([P, C], MM_DT, name=f"g{c1}")
                nc.vector.tensor_scalar_mul(g_t, pg, SCALE)
                g_s.append(g_t)

            # WvT tiles: [c partition chunk, h free]
            wvT_s = []
            for cc_ in range(CC):
                wvT = cpool.tile([P, H], MM_DT, name=f"wvT{cc_}")
                for hc in range(CC):
                    ptt = ptp.tile([P, P], F32, name="ptw", tag="pt")
                    nc.tensor.transpose(
                        ptt, wv_s[hc][:, cc_ * P : (cc_ + 1) * P], ident
                    )
                    nc.vector.tensor_copy(wvT[:, hc * P : (hc + 1) * P], ptt)
                wvT_s.append(wvT)

            for b in range(nb):
                # load x[b] -> 2 tiles [128, C]
                xa = []
                for tcc in range(TC):
                    xat = sb.tile([P, C], F32, name=f"xa{tcc}", tag=f"xa{tcc}")
                    nc.sync.dma_start(xat, x_d[b, tcc * P : (tcc + 1) * P, :])
                    xa.append(xat)

                # transpose x -> xT tiles [c chunk][128, T]
                xT = []
                for cc_ in range(CC):
                    xTt = sb.tile([P, T], MM_DT, name=f"xT{cc_}", tag=f"xT{cc_}")
                    xT.append(xTt)
                for tcc in range(TC):
                    for cc_ in range(CC):
                        ptt = ptp.tile([P, P], F32, name="ptx", tag="pt")
                        nc.tensor.transpose(
                            ptt, xa[tcc][:, cc_ * P : (cc_ + 1) * P], ident
                        )
                        nc.vector.tensor_copy(
                            xT[cc_][:, tcc * P : (tcc + 1) * P], ptt
                        )

                # z2[c2] = sum_c1 G[c1, c2-chunk] * xT[c1]   ([C, T], scaled)
                z2 = []
                for c2 in range(CC):
                    pz = pmp.tile([P, 512], F32, name="pz", tag="pm")[:, :T]
                    for c1 in range(CC):
                        nc.tensor.matmul(
                            pz,
                            lhsT=g_s[c1][:, c2 * P : (c2 + 1) * P],
                            rhs=xT[c1],
                            start=(c1 == 0),
                            stop=(c1 == CC - 1),
                        )
                    z2t = sb.tile([P, T], MM_DT, name=f"z2{c2}", tag=f"z2{c2}")
                    nc.vector.tensor_copy(z2t, pz)
                    z2.append(z2t)

                # v_aug[sc] = [x[b] @ Wv.T | 1]   ([128, H+1])
                vau = []
                for sc in range(TC):
                    pv = pmp.tile([P, 512], F32, name="pv", tag="pm")[:, :H]
                    for cc_ in range(CC):
                        nc.tensor.matmul(
                            pv,
                            lhsT=xT[cc_][:, sc * P : (sc + 1) * P],
                            rhs=wvT_s[cc_],
                            start=(cc_ == 0),
                            stop=(cc_ == CC - 1),
                        )
                    vt = sb.tile([P, H + 4], MM_DT, name=f"v{sc}", tag=f"v{sc}")
                    nc.vector.tensor_copy(vt[:, :H], pv)
                    # ones columns for the softmax-denominator trick (padded
                    # to 4 cols so the moving operand stays 16B-aligned),
                    # written via DVE so the values are f32r-rounded
                    nc.vector.tensor_scalar(
                        vt[:, H : H + 4],
                        masks[0][:, :4],
                        0.0,
                        1.0,
                        mybir.AluOpType.mult,
                        mybir.AluOpType.add,
                    )
                    vau.append(vt)

                # ST[s, t] = scaled scores transposed; exp + causal 0/1 mask
                est = []
                for sc in range(TC):
                    pst = pmp.tile([P, 512], F32, name="pst", tag="pm")[:, :T]
                    for cc_ in range(CC):
                        nc.tensor.matmul(
                            pst,
                            lhsT=xT[cc_][:, sc * P : (sc + 1) * P],
                            rhs=z2[cc_],
                            start=(cc_ == 0),
                            stop=(cc_ == CC - 1),
                        )
                    et = sb.tile([P, T], MM_DT, name=f"e{sc}", tag=f"e{sc}")
                    nc.scalar.activation(
                        et, pst, mybir.ActivationFunctionType.Exp
                    )
                    nc.vector.tensor_mul(et, et, masks[sc])
                    est.append(et)

                # out[t, h] = (sum_s est[s, t] * v_aug[s, h]) / r[t]
                for tcc in range(TC):
                    po = pmp.tile([P, 512], F32, name="po", tag="pm")[:, : H + 4]
                    for sc in range(TC):
                        nc.tensor.matmul(
                            po,
                            lhsT=est[sc][:, tcc * P : (tcc + 1) * P],
                            rhs=vau[sc],
                            start=(sc == 0),
                            stop=(sc == TC - 1),
                        )
                    rec = obp.tile([P, 1], F32, name="rec", tag="rec")
                    nc.vector.reciprocal(rec, po[:, H : H + 1])
                    ot = obp.tile([P, H], F32, name="ot", tag="ot")
                    nc.vector.tensor_scalar_mul(ot, po[:, :H], rec)
                    nc.sync.dma_start(out_d[b, tcc * P : (tcc + 1) * P, :], ot)

    nc.compile()
    return nc


_NC_CACHE = {}


def _get_nc(nb: int):
    if nb not in _NC_CACHE:
        _NC_CACHE[nb] = build_bass(nb)
    return _NC_CACHE[nb]


def kernel(x: np.ndarray, Wk: np.ndarray, Wq: np.ndarray, Wv: np.ndarray, **_):
    x = np.ascontiguousarray(x, dtype=np.float32)
    Wk = np.ascontiguousarray(Wk, dtype=np.float32)
    Wq = np.ascontiguousarray(Wq, dtype=np.float32)
    Wv = np.ascontiguousarray(Wv, dtype=np.float32)
    nb = x.shape[0] // NCORES
    nc = _get_nc(nb)
    in_maps = [
        {"x": x[i * nb : (i + 1) * nb], "Wk": Wk, "Wq": Wq, "Wv": Wv}
        for i in range(NCORES)
    ]
    res = run_bass_kernel_spmd(nc, in_maps, core_ids=list(range(NCORES)))
    return np.concatenate([r["out"] for r in res.results], axis=0)


if __name__ == "__main__":
    rng = np.random.default_rng(0)
    x = rng.standard_normal((B, T, C), dtype=np.float32)
    s = 1.0 / np.sqrt(C)
    Wk = rng.standard_normal((H, C), dtype=np.float32) * s
    Wq = rng.standard_normal((H, C), dtype=np.float32) * s
    Wv = rng.standard_normal((H, C), dtype=np.float32) * s
    out = kernel(x=x, Wk=Wk, Wq=Wq, Wv=Wv)
    print(out.shape, out.dtype)

